# revision 29
# baseline (speedup 1.0000x reference)
"""DAHHConv (hypergraph conv) Trainium2 Bass kernel, 8-core SPMD.

Math (reference):
    x' = x @ theta                      # [B,N,C]
    xe = (H^T x') / deg_e               # [B,E,C], deg_e = sum_n H
    xn = (H xe) / deg_n                 # [B,N,C], deg_n = sum_e H
    out = xn + bias

Sharding: 8 cores = 4 batches x 2 halves; core c -> batch b=c//2, half
h=c%2. Phase 1 (edge aggregation, contraction over n) shards E: each
core owns e in [1024h, 1024h+1024) with all N rows local (no
reduction). Phase 3 (node aggregation, contraction over e) shards N:
each core owns n in [4096h, 4096h+4096) and needs the full E range —
the locally-divided xe halves are exchanged with a 2-rank AllGather
(fp8 hi/lo payload).

Schedule (critical-path oriented):
  - phase 1 streams only hn (8MB); ht (8MB) streams AFTER, with the
    core's OWN e-chunks first (host pre-reorders ht rows own-first, so
    the program is SPMD-symmetric: chunk slots 0-7 are always local).
  - phase 2 runs on the OWN half only, pre-collective; the peer half
    arrives as ready-to-use xe. Own-half phase-3 matmuls for all four
    output spans run DURING the collective (ps3 bufs=4 = 8 PSUM banks).
  - the AllGather returns both ranks' blocks; the peer block is
    selected with a host-fed 0/1 mask (2 muls + add, exact in fp8),
    keeping the compiled program rank-agnostic.

Layout tricks:
  - H is fp8 (exact for 0/1) in both layouts; matmuls use DoubleRow
    perf mode with the stationary operand as an fp8 [hi|lo] pair
    (x' = hi+lo to bf16-level precision) and the moving H tile
    duplicated via a stride-0 AP dim.
  - deg_e / deg_n come free as a ones-column in the stationary operand.
  - xe is stored scaled by XES=64 (raw xe sits in fp8-subnormal range);
    1/XES is folded into the deg_n reciprocal chain.
  - hn rows are consumed in a host-permuted order (4 consecutive DRAM
    rows per SBUF partition) so every DMA line is 4KB contiguous.
  - phase 3 keeps y^T in [feature, node] layout; deg_n division uses a
    DRAM-hop repartition (reciprocal on 128 lanes) + stride-0 broadcast
    DMA; the output is written transposed and the host transposes back.
"""

import numpy as np
import ml_dtypes

B, N, E, C = 4, 8192, 2048, 64
NCORES = 8
EH = E // 2          # 1024: e-range per core in phase 1
NH = N // 2          # 4096: n-range per core in phase 3
CA = C + 1           # 65: feature dim augmented with ones/deg column
CW = 96              # DoubleRow weight width per k-subtile (multiple of 32)
CB = 2 * CW          # 192: fp8 [hi(65)+pad | lo(65)+pad] pair layout
XES = 64.0           # xe fp8 scale (power of 2; undone in the deg_n chain)
NCHUNK = N // 128    # 64 n-chunks in phase 1
HNTILES = N // 512   # 16 hn DMA tiles (512 rows each)
ECHUNK = E // 128    # 16 e-chunk slots in phase 3 (0-7 own, 8-15 peer)
NSPAN = 1024         # phase-3 output span (2 PSUM banks at fp32)
PK = 2 * CA * (ECHUNK // 2)  # 1040: packed xe bytes/partition per half
BF16 = ml_dtypes.bfloat16
FP8 = ml_dtypes.float8_e4m3

_cache = {}


def _split_waits_json(raw: bytes) -> bytes:
    """BIR post-pass: this walrus/ISA build allows only ONE sync wait per
    instruction, but the Tile scheduler attaches several. Hoist all but
    the last wait of each instruction onto standalone EventSemaphore
    instructions inserted just before it on the same engine (waits are
    pure preconditions, so running them earlier on the same engine
    stream is equivalent)."""
    import json

    m = json.loads(raw)
    ctr = 0
    for f in m["functions"]:
        for blk in f["blocks"]:
            new = []
            for inst in blk["instructions"]:
                si = inst.get("sync_info")
                waits = (si or {}).get("on_wait") or []
                if len(waits) > 1:
                    for w in waits[:-1]:
                        ctr += 1
                        new.append(
                            {
                                "debug": inst.get("debug", 0),
                                "engine": inst["engine"],
                                "ins": [],
                                "name": f"{inst['name']}-xw{ctr}",
                                "opcode": "EventSemaphore",
                                "outs": [],
                                "sync_info": {"on_update": [], "on_wait": [w]},
                            }
                        )
                    si["on_wait"] = [waits[-1]]
                new.append(inst)
            blk["instructions"] = new
    return json.dumps(m).encode()


def build_bass():
    import concourse.bass as bass
    import concourse.mybir as mybir
    from concourse.tile import TileContext
    from concourse import masks

    dt = mybir.dt
    nc = bass.Bass()

    hn = nc.declare_dram_parameter("hn", [N, EH], dt.float8e4, isOutput=False)
    ht = nc.declare_dram_parameter("ht", [E, NH], dt.float8e4, isOutput=False)
    xt = nc.declare_dram_parameter("xt", [C, N], dt.bfloat16, isOutput=False)
    th = nc.declare_dram_parameter("th", [C, C], dt.bfloat16, isOutput=False)
    pm = nc.declare_dram_parameter("pm", [128, 2], dt.float32, isOutput=False)
    out = nc.declare_dram_parameter("out", [C, NH], dt.float32, isOutput=True)

    # collective bounce buffers (DRAM; SBUF collectives are banned)
    cc_in = nc.dram_tensor("cc_in", [128, PK], dt.float8e4)
    cc_out = nc.dram_tensor("cc_out", [2 * 128, PK], dt.float8e4)
    # deg_n staging rows (DRAM hop: repartition + broadcast DMA)
    ddram = nc.dram_tensor("ddram", [NH // NSPAN, NSPAN], dt.float32)
    rdram = nc.dram_tensor("rdram", [NH // NSPAN, NSPAN], dt.float32)

    with TileContext(nc) as tc:
        with (
            tc.tile_pool(name="const", bufs=1) as const,
            tc.tile_pool(name="persist", bufs=1) as persist,
            tc.tile_pool(name="hn_pool", bufs=5) as hn_pool,
            tc.tile_pool(name="ht_pool", bufs=1) as ht_pool,
            tc.tile_pool(name="small", bufs=2) as small,
        ):
            ident = const.tile([128, 128], dt.float32)
            masks.make_identity(nc, ident[:])
            th_sb = const.tile([C, C], dt.bfloat16)
            nc.sync.dma_start(th_sb[:], th[:])
            pm_sb = const.tile([128, 2], dt.float32)
            nc.sync.dma_start(pm_sb[:], pm[:])
            xt_sb = persist.tile([C, N], dt.bfloat16)
            for q in range(4):
                nc.sync.dma_start(
                    xt_sb[:, 2048 * q : 2048 * (q + 1)],
                    xt[:, 2048 * q : 2048 * (q + 1)],
                )
            ht_tiles = [
                ht_pool.tile([128, NH], dt.float8e4, tag=f"ht{k}", name=f"ht{k}")
                for k in range(ECHUNK)
            ]

            # x'_aug chunks as fp8 hi/lo pairs: chunk j at cols
            # [192j, 192j+192) = [hi(65)+pad(31) | lo(65)+pad(31)]; hi
            # col C = 1 (ones for deg_e), lo col C = 0, pads zero. The
            # DoubleRow matmul computes hi^T@H + lo^T@H = x'^T@H.
            xp_sb = persist.tile([128, CB * NCHUNK], dt.float8e4)
            xp_v = xp_sb[:].rearrange("p (c two w) -> p c two w", two=2, w=CW)
            nc.vector.memset(xp_v[:, :, 0, C:CW], 0.0)
            nc.vector.memset(xp_v[:, :, 1, C:CW], 0.0)
            nc.vector.memset(xp_v[:, :, 0, C : C + 1], 1.0)

            # ---- phase 0: x' = x @ theta (theta stationary per chunk) ----
            with tc.tile_pool(name="ps0", bufs=2, space="PSUM") as ps0:
                for blk in range(NCHUNK // 8):
                    ps_xp = ps0.tile([128, 8 * C], dt.float32)
                    for jj in range(8):
                        j = 8 * blk + jj
                        nc.tensor.matmul(
                            ps_xp[:, C * jj : C * (jj + 1)],
                            xt_sb[:, 128 * j : 128 * (j + 1)],
                            th_sb[:],
                        )
                    src = ps_xp[:].rearrange("p (c w) -> p c w", w=C)
                    hi = xp_v[:, 8 * blk : 8 * (blk + 1), 0, 0:C]
                    nc.vector.tensor_copy(hi, src)
                    lo = xp_v[:, 8 * blk : 8 * (blk + 1), 1, 0:C]
                    nc.vector.tensor_tensor(
                        lo, src, hi, mybir.AluOpType.subtract
                    )

            # ---- phase 1: m_e^T[65,1024] = x'_aug^T @ H_n  (accum) ----
            # hn tile t covers DRAM rows [512t, 512t+512): partition p
            # holds rows 512t+4p..512t+4p+3 (4KB contiguous lines); the
            # matching x' chunks are j = 4t..4t+3 (xt is host-permuted).
            mr = small.tile([CA, EH], dt.float32, tag="mr")
            with tc.tile_pool(name="ps1", bufs=1, space="PSUM") as ps1:
                ps_me = ps1.tile([CW, EH], dt.float32)
                for t in range(HNTILES):
                    hn_t = hn_pool.tile([128, 4 * EH], dt.float8e4)
                    src = hn[512 * t : 512 * (t + 1), :].rearrange(
                        "(p four) e -> p (four e)", four=4
                    )
                    nc.sync.dma_start(hn_t[:], src)
                    for q in range(4):
                        j = 4 * t + q
                        for half in range(2):
                            rhs = hn_t[
                                :,
                                1024 * q + 512 * half : 1024 * q + 512 * (half + 1),
                            ]
                            nc.tensor.matmul(
                                ps_me[:, 512 * half : 512 * (half + 1)],
                                xp_sb[:, CB * j : CB * (j + 1)].rearrange(
                                    "p (two w) -> p two w", two=2
                                ),
                                rhs.unsqueeze(1).broadcast_to((128, 2, 512)),
                                start=(t == 0 and q == 0),
                                stop=(t == HNTILES - 1 and q == 3),
                                perf_mode=mybir.MatmulPerfMode.DoubleRow,
                            )
                nc.vector.tensor_copy(mr[:], ps_me[0:CA, :])

            # ---- phase 2 (own half only): xe = XES * m_e / deg_e ----
            xe_sb = persist.tile([128, CB * ECHUNK], dt.float8e4)
            xe_v = xe_sb[:].rearrange("p (c two w) -> p c two w", two=2, w=CW)
            nc.vector.memset(xe_v[:, :, 0, C:CW], 0.0)
            nc.vector.memset(xe_v[:, :, 1, C:CW], 0.0)
            nc.vector.memset(xe_v[:, 0:8, 0, C : C + 1], 1.0)
            with tc.tile_pool(name="ps2", bufs=2, space="PSUM") as ps2:
                for t in range(ECHUNK // 2):
                    ps_tr = ps2.tile([128, CA], dt.float32)
                    nc.tensor.transpose(
                        ps_tr[:], mr[:, 128 * t : 128 * (t + 1)], ident[0:CA, 0:CA]
                    )
                    rec = small.tile([128, 1], dt.float32, tag="rec")
                    nc.vector.reciprocal(rec[:], ps_tr[:, C : C + 1])
                    srec = small.tile([128, 1], dt.float32, tag="srec")
                    nc.vector.tensor_scalar_mul(srec[:], rec[:], float(XES))
                    t32 = small.tile([128, C], dt.float32, tag="t32")
                    nc.vector.tensor_scalar_mul(t32[:], ps_tr[:, 0:C], srec[:])
                    hi = xe_v[:, t, 0, 0:C]
                    nc.vector.tensor_copy(hi, t32[:])
                    lo = xe_v[:, t, 1, 0:C]
                    nc.vector.tensor_tensor(
                        lo, t32[:], hi, mybir.AluOpType.subtract
                    )

            # pack own xe (hi/lo, no pads) and exchange within the pair
            cc_in_v = cc_in[:].rearrange("p (k two w) -> p k two w", two=2, w=CA)
            nc.sync.dma_start(cc_in_v, xe_v[:, 0:8, :, 0:CA])
            nc.gpsimd.collective_compute(
                "AllGather",
                mybir.AluOpType.bypass,
                replica_groups=[[0, 1], [2, 3], [4, 5], [6, 7]],
                ins=[cc_in[:]],
                outs=[cc_out[:]],
            )

            # ht streams after hn, own e-chunk slots 0-7 first. The
            # triggers go on the GpSimd queue: the Sync queue's in-order
            # head-of-line blocking would otherwise delay the collective
            # pack/unpack DMAs behind 8MB of ht traffic.
            for k in range(ECHUNK):
                nc.gpsimd.dma_start(ht_tiles[k][:], ht[128 * k : 128 * (k + 1), :])
            # select the peer's block with the host-fed 0/1 mask:
            # peer = m0*block0 + m1*block1 (exact: one term is zero)
            b0 = small.tile([128, PK], dt.float8e4, tag="b0")
            nc.sync.dma_start(b0[:], cc_out[0:128, :])
            b1 = small.tile([128, PK], dt.float8e4, tag="b1")
            nc.sync.dma_start(b1[:], cc_out[128:256, :])
            t0 = small.tile([128, PK], dt.bfloat16, tag="t0")
            nc.vector.tensor_scalar_mul(t0[:], b0[:], pm_sb[:, 0:1])
            t1 = small.tile([128, PK], dt.bfloat16, tag="t1")
            nc.vector.tensor_scalar_mul(t1[:], b1[:], pm_sb[:, 1:2])
            pv = "p (k two w) -> p k two w"
            nc.vector.tensor_tensor(
                xe_v[:, 8:16, :, 0:CA],
                t0[:].rearrange(pv, two=2, w=CA),
                t1[:].rearrange(pv, two=2, w=CA),
                mybir.AluOpType.add,
            )

            # ---- phase 3: y^T[65,span] = xe_aug^T @ H_e^T; out = y/deg_n ----
            # own chunk slots (0-7) for ALL four spans run during the
            # collective window; peer slots (8-15) after, span-major so
            # span completions stagger and posts pipeline.
            with tc.tile_pool(name="ps3", bufs=4, space="PSUM") as ps3:
                span_ps = {
                    s: ps3.tile([CW, NSPAN], dt.float32, tag="ps_y", name=f"ps_y{s}")
                    for s in range(4)
                }

                def span_mm(s, k):
                    for half in range(2):
                        col = NSPAN * s + 512 * half
                        nc.tensor.matmul(
                            span_ps[s][:, 512 * half : 512 * (half + 1)],
                            xe_sb[:, CB * k : CB * (k + 1)].rearrange(
                                "p (two w) -> p two w", two=2
                            ),
                            ht_tiles[k][:, col : col + 512]
                            .unsqueeze(1)
                            .broadcast_to((128, 2, 512)),
                            start=(k == 0),
                            stop=(k == ECHUNK - 1),
                            perf_mode=mybir.MatmulPerfMode.DoubleRow,
                        )

                def span_post(s):
                    ps_y = span_ps[s]
                    # deg row: DMA straight from PSUM -> DRAM -> [128,8]
                    # so the reciprocal runs on 128 DVE lanes (13 cyc/elem
                    # on one lane otherwise). Triggers go on the Scalar
                    # queue to dodge Sync-queue head-of-line blocking.
                    drow = small.tile([1, NSPAN], dt.float32, tag="drow")
                    nc.vector.tensor_copy(drow[:], ps_y[C : C + 1, :])
                    nc.scalar.dma_start(ddram[s : s + 1, :], drow[:])
                    dcol = small.tile([128, NSPAN // 128], dt.float32, tag="dcol")
                    nc.scalar.dma_start(
                        dcol[:],
                        ddram[s : s + 1, :].rearrange("one (p f) -> (one p) f", p=128),
                    )
                    rcol = small.tile([128, NSPAN // 128], dt.float32, tag="rcol")
                    nc.vector.reciprocal(rcol[:], dcol[:])
                    srcol = small.tile([128, NSPAN // 128], dt.float32, tag="srcol")
                    nc.vector.tensor_scalar_mul(srcol[:], rcol[:], 1.0 / float(XES))
                    nc.scalar.dma_start(
                        rdram[s : s + 1, :].rearrange("one (p f) -> (one p) f", p=128),
                        srcol[:],
                    )
                    rrep = small.tile([C, NSPAN], dt.float32, tag="rrep")
                    bcast_src = bass.AP(
                        tensor=rdram,
                        offset=s * NSPAN,
                        ap=[[0, C], [1, NSPAN]],
                    )
                    nc.scalar.dma_start(rrep[:], bcast_src)
                    o_sb = small.tile([C, NSPAN], dt.float32, tag="o_sb")
                    nc.vector.tensor_tensor(
                        o_sb[:], ps_y[0:C, :], rrep[:], mybir.AluOpType.mult
                    )
                    nc.scalar.dma_start(
                        out[:, NSPAN * s : NSPAN * (s + 1)], o_sb[:]
                    )

                for k in range(8):
                    for s in range(4):
                        span_mm(s, k)
                for s in range(4):
                    for k in range(8, ECHUNK):
                        span_mm(s, k)
                    span_post(s)

    orig_to_json = nc.to_json_bytes
    nc.to_json_bytes = lambda: _split_waits_json(orig_to_json())
    return nc


def _fp8_exact(a):
    # H is 0/1: 1.0 is exactly 0x38 in float8_e4m3.
    return (np.where(a != 0, 0x38, 0)).astype(np.uint8).view(FP8)


def _prepare_in_maps(x, H, theta):
    x = np.ascontiguousarray(x, dtype=np.float32)
    H = np.ascontiguousarray(H, dtype=np.float32)
    th16 = np.ascontiguousarray(theta, dtype=np.float32).astype(BF16)
    in_maps = []
    for c in range(NCORES):
        b, h = divmod(c, 2)
        hn = _fp8_exact(np.ascontiguousarray(H[b, :, EH * h : EH * (h + 1)]))
        # ht rows reordered own-e-range first so chunk slots 0-7 are
        # always the core's own half (SPMD-symmetric program).
        htf = H[b, NH * h : NH * (h + 1), :].T
        own = htf[EH * h : EH * (h + 1), :]
        peer = htf[EH * (1 - h) : EH * (1 - h) + EH, :]
        ht = _fp8_exact(np.ascontiguousarray(np.concatenate([own, peer], axis=0)))
        # phase-1 consumes n in blocks of 512 as [128 partitions x 4 rows]:
        # chunk j = 4t+q, partition p <-> DRAM row 512t+4p+q. Permute xt's
        # columns to match (the n-contraction is order-invariant).
        xtb = x[b].T.reshape(C, HNTILES, 128, 4)
        xtp = np.ascontiguousarray(
            xtb.transpose(0, 1, 3, 2).reshape(C, N)
        ).astype(BF16)
        pmv = np.zeros((128, 2), dtype=np.float32)
        pmv[:, 1 - h] = 1.0
        in_maps.append({"hn": hn, "ht": ht, "xt": xtp, "th": th16, "pm": pmv})
    return in_maps


def _assemble(results, bias):
    out = np.empty((B, N, C), dtype=np.float32)
    for c in range(NCORES):
        b, h = divmod(c, 2)
        out[b, NH * h : NH * (h + 1), :] = results[c]["out"].T
    out += np.asarray(bias, dtype=np.float32)[None, None, :]
    return out


def get_nc():
    if "nc" not in _cache:
        _cache["nc"] = build_bass()
    return _cache["nc"]


def kernel(x, H, theta, bias):
    from concourse.bass_utils import run_bass_kernel_spmd

    nc = get_nc()
    in_maps = _prepare_in_maps(x, H, theta)
    res = run_bass_kernel_spmd(nc, in_maps, list(range(NCORES)))
    return _assemble(res.results, bias)
